# revision 17
# baseline (speedup 1.0000x reference)
# Trainium2 Bass kernel for nn_Attention_80779744903426
#
# Reference computation (b=4, n=2048, c=1024, h=16, d=64):
#   qkv = x @ w_qkv ; split to q,k,v per head
#   attn = softmax(q k^T / sqrt(c)) ; out = (attn v) concat ; y = out @ w_proj + b_proj
#
# Sharding (8 cores): data-parallel over batch (4) x tensor-parallel over
# head-groups (2 groups of 8 heads, Megatron-style). Each core computes a
# partial y for its batch from its 8 heads; host sums the two partials per
# batch and adds b_proj.
#
# Engine plan (per core). PE throughput law (measured): time = moving-stream
# elems/partition x 0.42ns; 64-row-tiled matmul pairs execute CONCURRENTLY,
# so the d=64 S matmul pairs run 2 heads in the time of one (107ns each).
# The softmax exp (33.5M elems/core) saturates the scalar(ACT) engine alone
# (218us), so it is split between ACT Exp and a custom 8-stage DVE op
# computing exp(s*SCALE) = [p3(s*SCALE/4)]^4 (cubic Horner + two squarings,
# max rel err 2.4e-3, scale folded into the coefficients).
#
# Schedule: 16 fine slots per (pair, q-chunk) iteration, one attention
# k-tile per slot: S pair -> exp pair (ACT||DVE) -> PV k-tile of the
# previous iteration -> optional extra (projection chain / y chain).
# All cross-engine queues are kept shallow so no in-order engine queue
# head-of-line-blocks the exp stream: y staging copies are stashed and
# flushed one slot later on ACT; the softmax-denominator broadcast rides
# the gpsimd DMA queue; the normalization muls are emitted only after the
# bounce has had two slots of lead time.

import numpy as np

DIM = 1024
N = 2048
B = 4
NH = 16
HD = 64
SCALE = 1.0 / DIM**0.5

HPC = 8            # heads per core
PAIRS = HPC // 2   # head pairs (row-tiled together)
CT = 8             # contraction tiles over c=1024
ACH = 512          # phase-A n-chunk
QCH = 512          # phase-B q-chunk
NQC = N // QCH     # 4 q-chunks
KT = 16            # k tiles of 128 in attention

# cubic minimax fit of exp(u) on |u|<=0.55, c0 pinned to 1 (DVE `One` leaf);
# exp(x) = p(x/4)^4. Constants folded with SCALE: s = raw logit.
_PC1, _PC2, _PC3 = 1.0012104626026934, 0.5103362584310798, 0.163023563657408
_K4 = SCALE / 4.0
EXP_S0 = _PC1 * _K4
EXP_S1 = _PC2 * _K4 * _K4
EXP_IMM2 = _PC3 * _K4 * _K4 * _K4

# groups (of 8) whose head-B exp chunk runs on ACT instead of DVE; early
# groups stay on ACT because the DVE ends each iteration with norm work
ACT_HH1_GROUPS = {0, 1}

_CACHE = {}


def _register_exp_op():
    """Custom DVE op EXP_POLY4_ANT: sq(sq(1 + s*(C0 + s*(C1 + s*C2)))).
    8/8 v3 stages, one pass, fp32 in (SBUF or PSUM), bf16 out."""
    from concourse import dve_ops as _dve_ops
    from concourse.dve_spec import (
        Spec, Src0, One, sq, lower,
        C0 as LC0, C1 as LC1, C2 as LC2,
    )
    from concourse.dve_uop import DveOpSpec

    name = "EXP_POLY4_ANT"
    for op in _dve_ops.OPS:
        if op.name == name:
            return op

    def ref(in0, in1, s0, s1, imm2):
        p = 1.0 + in0.astype(np.float32) * (s0 + in0 * (s1 + in0 * imm2))
        return ((p * p) * (p * p)).astype(np.float32)

    spec = Spec(
        body=sq(sq(One + Src0 * (LC0 + Src0 * (LC1 + Src0 * LC2)))),
        reference=ref,
    )
    shas = {}
    for ver in ("v3", "v4"):
        shas[ver] = DveOpSpec(name=name, uops=lower(spec, ver=ver)).sha(ver)
    op = _dve_ops.DveOp(name, spec, subdim=False, uops_sha=shas)
    _dve_ops.OPS.append(op)
    _dve_ops.CUSTOM_DVE_SPECS[name] = spec
    _dve_ops._SUB_OPCODE_FOR_NAME[name] = (
        _dve_ops._CUSTOM_DVE_ROW_BASE + len(_dve_ops.OPS) - 1
    )
    return op


def _build_nc():
    import concourse.bass as bass
    from concourse import bacc, mybir, tile

    f32 = mybir.dt.float32
    bf16 = mybir.dt.bfloat16
    EXP = mybir.ActivationFunctionType.Exp
    COPY = mybir.ActivationFunctionType.Copy

    exp_op = _register_exp_op()

    nc = bacc.Bacc("TRN2", target_bir_lowering=False, debug=False)

    xT_d = nc.dram_tensor("xT", [DIM, N], bf16, kind="ExternalInput").ap()
    wqk_d = nc.dram_tensor("wqk", [DIM, 1024], bf16, kind="ExternalInput").ap()
    wv_d = nc.dram_tensor("wv", [DIM, 512], bf16, kind="ExternalInput").ap()
    wp_d = nc.dram_tensor("wp", [512, DIM], bf16, kind="ExternalInput").ap()
    y_d = nc.dram_tensor("y", [N, DIM], f32, kind="ExternalOutput").ap()

    with tile.TileContext(nc) as tc:
        with (
            tc.tile_pool(name="pt", bufs=4) as ptp,       # 16KB ptile slots
            tc.tile_pool(name="xt", bufs=2) as xtp,       # 8KB xt chunks
            tc.tile_pool(name="wqk", bufs=1) as wqkp,
            tc.tile_pool(name="wv", bufs=1) as wvp,
            tc.tile_pool(name="wp", bufs=1) as wpp,
            tc.tile_pool(name="v", bufs=1) as vp,
            tc.tile_pool(name="ot", bufs=1) as otp,
            tc.tile_pool(name="misc", bufs=2) as miscp,
            tc.tile_pool(name="ps", bufs=1, space="PSUM") as psp,
            tc.tile_pool(name="dram", bufs=1, space="DRAM") as dp,
        ):
            # ---- static tiles (wp loads deferred past pass 1: first
            # needed at iter 5; keeps the startup DMA queue short) ----
            wqk_sb = wqkp.tile([128, CT, 1024], bf16)
            wv_sb = wvp.tile([128, CT, 512], bf16)
            wp_sb = wpp.tile([128, 4, 1024], bf16)

            v_sb = vp.tile([128, KT, HPC, HD + 1], bf16)  # [k-part, k-tile, head, d | 1]
            nc.vector.memset(v_sb[:, :, :, HD], 1.0)

            ot_sb = otp.tile([128, PAIRS, N], bf16)  # O^T rows: pair p = rows 128p..
            qt_all = otp.tile([128, 4, N], bf16, name="qt_all")
            kt_all = otp.tile([128, 4, N], bf16, name="kt_all")

            xT_r = xT_d.rearrange("(t p) n -> p t n", p=128)

            # ---- phase A helpers ----
            def emit_qkt_chains(jobs):
                for xt, mt, ach in jobs:
                    qps = psp.tile([128, 512], f32, tag="acc", bufs=2, name="qps")
                    for ct in range(CT):
                        nc.tensor.matmul(
                            qps, wqk_sb[:, ct, 128 * mt : 128 * (mt + 1)],
                            xt[:, ct, :], start=(ct == 0), stop=(ct == CT - 1))
                    dst = qt_all if mt < 4 else kt_all
                    nc.vector.tensor_copy(
                        dst[:, mt % 4, ACH * ach : ACH * (ach + 1)], qps)

            def load_xt(ach):
                xt = xtp.tile([128, CT, ACH], bf16, tag="xt", bufs=2, name="xt")
                nc.sync.dma_start(xt, xT_r[:, :, ACH * ach : ACH * (ach + 1)])
                return xt

            def emit_v_group_on(xt, ach):
                for sub in range(ACH // 128):
                    nt = (ACH // 128) * ach + sub
                    vps = psp.tile([128, 512], f32, tag="acc", bufs=2, name="vps")
                    for ct in range(CT):
                        nc.tensor.matmul(vps, xt[:, ct, 128 * sub : 128 * (sub + 1)],
                                         wv_sb[:, ct, :], start=(ct == 0),
                                         stop=(ct == CT - 1))
                    nc.vector.tensor_copy(
                        v_sb[:, nt, :, 0:HD],
                        vps.rearrange("p (h d) -> p h d", h=HPC),
                    )

            # ---- phase A pass 1: K^T pair0 + V + Q^T(pair0, chunk0) ----
            # sequential per chunk so the xt pool double-buffers (DMA of
            # chunk a+1 hides under chunk a's chains). xt0 load is emitted
            # before the weight DMAs so the PE starts ~3us in.
            xt0 = load_xt(0)
            for ct in range(CT):
                nc.sync.dma_start(wqk_sb[:, ct, :], wqk_d[128 * ct : 128 * (ct + 1), :])
            for ct in range(CT):
                nc.sync.dma_start(wv_sb[:, ct, :], wv_d[128 * ct : 128 * (ct + 1), :])
            for ach in range(N // ACH):
                xt = xt0 if ach == 0 else load_xt(ach)
                emit_qkt_chains([(xt, 4, ach)])
                if ach == 0:
                    emit_qkt_chains([(xt, 0, 0)])
                emit_v_group_on(xt, ach)
            for ot in range(4):
                nc.sync.dma_start(wp_sb[:, ot, :], wp_d[128 * ot : 128 * (ot + 1), :])

            # ---- y projection: one 4-matmul chain per extra slot, staging
            # copy stashed and flushed a slot later (on ACT) so neither the
            # ACT queue nor the PSUM pool ever blocks the exp stream.
            pending_ystg = []

            def flush_ystg():
                while pending_ystg:
                    yps, nt2, yc = pending_ystg.pop(0)
                    stg = miscp.tile([128, 512], f32, tag="ystg", bufs=2,
                                     name="ystg")
                    nc.scalar.activation(out=stg, in_=yps, func=COPY)
                    nc.sync.dma_start(
                        y_d[128 * nt2 : 128 * (nt2 + 1), 512 * yc : 512 * (yc + 1)],
                        stg,
                    )

            def emit_proj_chain(qc0, j):
                flush_ystg()
                nt2 = 4 * qc0 + j // 2
                yc = j % 2
                yps = psp.tile([128, 512], f32, tag="acc", bufs=2, name="yps")
                for ot in range(4):
                    nc.tensor.matmul(
                        yps, ot_sb[:, ot, 128 * nt2 : 128 * (nt2 + 1)],
                        wp_sb[:, ot, 512 * yc : 512 * (yc + 1)],
                        start=(ot == 0), stop=(ot == 3))
                pending_ystg.append((yps, nt2, yc))

            def proj_pair_thunk(qc0, j):
                def t():
                    emit_proj_chain(qc0, j)
                    emit_proj_chain(qc0, j + 1)
                return t

            def qkt_thunk(ach, mts):
                def t():
                    xt = load_xt(ach)
                    emit_qkt_chains([(xt, mt, ach) for mt in mts])
                return t

            # ---- per-iteration extras: one thunk per 2-slot group (8/iter).
            # Deadlines: K^T pair p before iter p; Q^T (mt p, chunk qc) before
            # iter 4qc+p; proj chains for qc after iter 4qc+4's norm.
            EXTRAS = {
                0: {0: qkt_thunk(0, [5, 1]), 2: qkt_thunk(1, [5]),
                    4: qkt_thunk(2, [5]), 6: qkt_thunk(3, [5])},
                1: {0: qkt_thunk(0, [6, 2]), 2: qkt_thunk(1, [6]),
                    4: qkt_thunk(2, [6]), 6: qkt_thunk(3, [6])},
                2: {0: qkt_thunk(0, [7, 3]), 2: qkt_thunk(1, [7]),
                    4: qkt_thunk(2, [7]), 6: qkt_thunk(3, [7])},
                3: {1: qkt_thunk(1, [0, 1]), 5: qkt_thunk(1, [2, 3])},
                4: {1: qkt_thunk(2, [0, 1]), 5: qkt_thunk(2, [2, 3])},
                5: {0: qkt_thunk(3, [0, 1]), 1: qkt_thunk(3, [2, 3]),
                    **{2 + j // 2: proj_pair_thunk(0, j) for j in range(0, 8, 2)}},
                9: {2 + j // 2: proj_pair_thunk(1, j) for j in range(0, 8, 2)},
                13: {2 + j // 2: proj_pair_thunk(2, j) for j in range(0, 8, 2)},
            }

            def emit_exp(eng, hh, g, sps, ptiles):
                if eng == "dve":
                    nc.vector._custom_dve(
                        exp_op,
                        out=ptiles[hh][:, 2 * g : 2 * g + 2, :],
                        in0=sps[hh],
                        s0=EXP_S0, s1=EXP_S1, imm2=EXP_IMM2,
                    )
                else:
                    nc.scalar.activation(
                        out=ptiles[hh][:, 2 * g : 2 * g + 2, :],
                        in_=sps[hh],
                        func=EXP,
                        scale=float(SCALE),
                    )

            def emit_pv_ktile(st, k):
                p0, ptl, opsl = st
                for hh in range(2):
                    h = 2 * p0 + hh
                    nc.tensor.matmul(opsl[hh], v_sb[:, k, h, :],
                                     ptl[hh][:, k, :],
                                     start=(k == 0), stop=(k == KT - 1))

            # front-load the 16 PV k-tiles into groups 0..5 so the
            # denominator (norm head) can issue at group 5 and the bounce
            # broadcast has ~2 groups of lead before the norm muls at group 7
            PV_GROUP_KTILES = {0: (0, 1, 2), 1: (3, 4, 5), 2: (6, 7, 8),
                               3: (9, 10, 11), 4: (12, 13), 5: (14, 15)}

            def emit_norm_head(st):
                # den copies + reciprocals + bounce DMAs (gpsimd queue)
                p0, ptl, opsl = st
                bcs = []
                for hh in range(2):
                    den = miscp.tile([1, QCH], f32, tag="den", bufs=2, name="den")
                    nc.vector.tensor_copy(den, opsl[hh][HD : HD + 1, :])
                    rcp = miscp.tile([1, QCH], f32, tag="rcp", bufs=2, name="rcp")
                    nc.vector.reciprocal_approx_fast(rcp, den)
                    rcp_d = dp.tile([1, QCH], f32, tag="rcpd", bufs=4, name="rcpd")
                    nc.gpsimd.dma_start(rcp_d, rcp)
                    bc = miscp.tile([64, QCH], f32, tag="bc", bufs=3, name="bc")
                    rap = rcp_d[:]
                    nc.gpsimd.dma_start(
                        bc,
                        bass.AP(tensor=rap.tensor, offset=rap.offset,
                                ap=[[0, 64]] + list(rap.ap[1:])),
                    )
                    bcs.append(bc)
                return bcs

            def emit_norm_tail(st, qc0, bcs):
                # normalize straight out of PSUM into ot_sb (DVE muls)
                p0, ptl, opsl = st
                for hh in range(2):
                    nc.vector.tensor_mul(
                        ot_sb[64 * hh : 64 * (hh + 1), p0, QCH * qc0 : QCH * (qc0 + 1)],
                        opsl[hh][0:HD, :],
                        bcs[hh],
                    )

            pv_st = None
            pv_qc = None
            it = -1
            for qc in range(NQC):
                for p in range(PAIRS):
                    it += 1
                    kt_sb = kt_all[:, p, :]
                    qt_sb = qt_all[:, p, QCH * qc : QCH * (qc + 1)]
                    extras = EXTRAS.get(it, {})
                    ptiles = [
                        ptp.tile([128, KT, QCH], bf16, tag="pt", bufs=4,
                                 name=f"pt{hh}")
                        for hh in range(2)
                    ]
                    bcs = None
                    for g in range(KT // 2):
                        sps = [
                            psp.tile([128, 2, QCH], f32, tag="sb2", bufs=2,
                                     name=f"sps{hh}")
                            for hh in range(2)
                        ]
                        for j in range(2):
                            k = 2 * g + j
                            for hh in range(2):
                                sl = slice(64 * hh, 64 * (hh + 1))
                                nc.tensor.matmul(
                                    sps[hh][:, j, :],
                                    kt_sb[sl, 128 * k : 128 * (k + 1)],
                                    qt_sb[sl, :], start=True, stop=True)
                        emit_exp("act", 0, g, sps, ptiles)
                        emit_exp("act" if g in ACT_HH1_GROUPS else "dve",
                                 1, g, sps, ptiles)
                        if pv_st is not None:
                            for kk in PV_GROUP_KTILES.get(g, ()):
                                emit_pv_ktile(pv_st, kk)
                            if g == 5:
                                bcs = emit_norm_head(pv_st)
                            elif g == 7:
                                emit_norm_tail(pv_st, pv_qc, bcs)
                        if g in extras:
                            extras[g]()
                    flush_ystg()
                    opsl = [
                        psp.tile([HD + 1, QCH], f32, tag="ops", bufs=2,
                                 name=f"ops{hh}")
                        for hh in range(2)
                    ]
                    pv_st = (p, ptiles, opsl)
                    pv_qc = qc
            # drain the last (qc3, pair3)
            for k in range(KT):
                emit_pv_ktile(pv_st, k)
            bcs = emit_norm_head(pv_st)
            emit_norm_tail(pv_st, pv_qc, bcs)
            for j in range(8):
                emit_proj_chain(3, j)
            flush_ystg()

    nc.compile()
    return nc


def get_nc():
    if "nc" not in _CACHE:
        _CACHE["nc"] = _build_nc()
    return _CACHE["nc"]


def make_in_maps(x, w_qkv, w_proj):
    import ml_dtypes

    bf = ml_dtypes.bfloat16
    in_maps = []
    for c in range(8):
        b, g = c // 2, c % 2
        in_maps.append({
            "xT": np.ascontiguousarray(x[b].T).astype(bf),
            "wqk": np.ascontiguousarray(
                np.concatenate(
                    [w_qkv[:, 512 * g : 512 * (g + 1)],
                     w_qkv[:, 1024 + 512 * g : 1024 + 512 * (g + 1)]], axis=1
                )).astype(bf),
            "wv": np.ascontiguousarray(
                w_qkv[:, 2048 + 512 * g : 2048 + 512 * (g + 1)]).astype(bf),
            "wp": np.ascontiguousarray(
                w_proj[512 * g : 512 * (g + 1), :]).astype(bf),
        })
    return in_maps


def kernel(x, w_qkv, w_proj, b_proj):
    from concourse.bass_utils import run_bass_kernel_spmd

    x = np.asarray(x, dtype=np.float32)
    w_qkv = np.asarray(w_qkv, dtype=np.float32)
    w_proj = np.asarray(w_proj, dtype=np.float32)
    b_proj = np.asarray(b_proj, dtype=np.float32)

    nc = get_nc()
    in_maps = make_in_maps(x, w_qkv, w_proj)
    res = run_bass_kernel_spmd(nc, in_maps, list(range(8))).results

    out = np.zeros((B, N, DIM), dtype=np.float32)
    for c in range(8):
        out[c // 2] += res[c]["y"]
    return out + b_proj
